# revision 1
# baseline (speedup 1.0000x reference)
"""Trainium2 Bass kernel for the scatter_memory GRU memory-update module.

Computation (torch GRUCell semantics, chunk order r, z, n):
    current = memory[node_ids]                       # [B, H] gather
    gi = messages @ W_ih.T + b_ih ; gh = current @ W_hh.T + b_hh
    r = sigmoid(gi_r + gh_r) ; z = sigmoid(gi_z + gh_z)
    n = tanh(gi_n + r * gh_n)
    updated = (1 - z) * n + z * current
    new_memory = memory.at[node_ids].set(updated)    # scatter

Distribution: the B updated rows are sharded contiguously across 8
NeuronCores.  The gather/scatter over the 500k-row table and the
feature-major transposes run on the host; each core runs the GRU math on
its own [H, B/8] shard (feature dim H=128 sits on the SBUF partition
axis, so the GRU biases become per-partition vectors that fuse into the
ScalarEngine activation ops for free).
"""

import os
import sys

import numpy as np

for _p in ("/opt/trn_rl_repo", "/root/.axon_site/_ro/trn_rl_repo"):
    if os.path.isdir(_p) and _p not in sys.path:
        sys.path.insert(0, _p)

import ml_dtypes
from contextlib import ExitStack

import concourse.bass as bass
import concourse.tile as tile
from concourse import mybir
from concourse.bass_utils import run_bass_kernel_spmd

BF16 = ml_dtypes.bfloat16
import json as _json

N_CORES = 8
H = 128
NTILE = 1024         # batch columns per PSUM tile (2 banks of fp32 per gate)
CHUNK = 2048         # batch columns per DMA chunk / wide elementwise ops

# exposed for test harnesses
LAST_RESULT = None

_NC_CACHE = {}


def _split_sync_waits(bir: dict) -> dict:
    """Hoist extra per-instruction semaphore waits into standalone
    EventSemaphore instructions.

    The walrus build in this container encodes at most ONE sync wait per
    instruction ("Too many sync wait commands" otherwise); Tile attaches
    one wait per dependency.  An engine-level standalone wait immediately
    before the instruction is semantically identical (the engine stalls
    either way), so keep the last wait inline and hoist the rest.
    """
    n = 0
    for fn in bir.get("functions", []):
        for blk in fn.get("blocks", []):
            out = []
            for inst in blk.get("instructions", []):
                si = inst.get("sync_info") or {}
                ow = si.get("on_wait") or []
                if len(ow) > 1:
                    for w in ow[:-1]:
                        n += 1
                        out.append({
                            "debug": inst.get("debug", 0),
                            "engine": inst["engine"],
                            "ins": [],
                            "outs": [],
                            "name": f"hoistw_{n}_{inst['name']}",
                            "opcode": "EventSemaphore",
                            "sync_info": {"on_update": [], "on_wait": [w]},
                        })
                    si["on_wait"] = [ow[-1]]
                out.append(inst)
            blk["instructions"] = out
    return bir


def _patch_json(nc: bass.Bass) -> None:
    orig = nc.to_json_bytes

    def patched() -> bytes:
        return _json.dumps(_split_sync_waits(_json.loads(orig()))).encode()

    nc.to_json_bytes = patched


def _build_nc(bpc: int) -> bass.Bass:
    """Bass program for one core: GRU over a [H, bpc] feature-major shard."""
    assert bpc % 512 == 0 and CHUNK % NTILE == 0
    f32 = mybir.dt.float32
    bf16 = mybir.dt.bfloat16
    sig = mybir.ActivationFunctionType.Sigmoid
    tanh = mybir.ActivationFunctionType.Tanh
    add_op = mybir.AluOpType.add
    mult_op = mybir.AluOpType.mult

    nc = bass.Bass()
    xT = nc.declare_dram_parameter("xT", [H, bpc], bf16, isOutput=False)
    hT = nc.declare_dram_parameter("hT", [H, bpc], bf16, isOutput=False)
    w_ihT = nc.declare_dram_parameter("w_ihT", [H, 3 * H], bf16, isOutput=False)
    w_hhT = nc.declare_dram_parameter("w_hhT", [H, 3 * H], bf16, isOutput=False)
    # bias columns: 0 = b_ih_r + b_hh_r, 1 = b_ih_z + b_hh_z, 2 = b_hh_n, 3 = b_ih_n
    biases = nc.declare_dram_parameter("biases", [H, 4], f32, isOutput=False)
    outT = nc.declare_dram_parameter("outT", [H, bpc], bf16, isOutput=True)

    with ExitStack() as ctx:
        tc = ctx.enter_context(tile.TileContext(nc))
        singles = ctx.enter_context(tc.tile_pool(name="singles", bufs=1))
        # enough io buffers that every chunk's x/h DMA can be issued up
        # front — the DMA queues stream while compute follows behind
        io = ctx.enter_context(tc.tile_pool(name="io", bufs=8))
        wide = ctx.enter_context(tc.tile_pool(name="wide", bufs=2))
        mids = ctx.enter_context(tc.tile_pool(name="mids", bufs=4))
        # 4 PSUM tensors x [128, 1024] fp32 = 2 banks each -> all 8 banks
        psum = ctx.enter_context(tc.tile_pool(name="psum", bufs=1, space="PSUM"))

        # small one-time loads go through SWDGE (single queue) so consumers
        # don't accumulate one sem wait per HWDGE hardware queue
        w_ih_sb = singles.tile([H, 3 * H], bf16)
        nc.gpsimd.dma_start(out=w_ih_sb, in_=w_ihT[:, :])
        w_hh_sb = singles.tile([H, 3 * H], bf16)
        nc.gpsimd.dma_start(out=w_hh_sb, in_=w_hhT[:, :])
        b_sb = singles.tile([H, 4], f32)
        nc.gpsimd.dma_start(out=b_sb, in_=biases[:, :])

        # dummy sigmoid fires the ~2.7us ACT table load immediately, so it
        # overlaps the DMA ramp instead of stalling the first real sigmoid
        warm_sb = singles.tile([H, 1], f32)
        nc.scalar.activation(out=warm_sb, in_=b_sb[:, 0:1],
                             func=sig, bias=0.0, scale=1.0)

        # small first chunk so compute starts before the first big DMA
        # lands; tapered last chunks so the final serial chain is short
        chunks = []
        pos = 0
        first = min(512, bpc)
        chunks.append((0, first))
        pos = first
        while bpc - pos > CHUNK:
            chunks.append((pos, CHUNK))
            pos += CHUNK
        for tail in (1024, 512, 512):
            if pos >= bpc:
                break
            csz = min(tail, bpc - pos)
            chunks.append((pos, csz))
            pos += csz
        assert pos == bpc, (pos, bpc, chunks)

        for ci, (c0, csz) in enumerate(chunks):
            x_sb = io.tile([H, csz], bf16, tag="x")
            h_sb = io.tile([H, csz], bf16, tag="h")
            nc.sync.dma_start(out=x_sb, in_=xT[:, c0 : c0 + csz])
            nc.sync.dma_start(out=h_sb, in_=hT[:, c0 : c0 + csz])
            o_sb = wide.tile([H, csz], bf16, tag="o", bufs=2)
            pre_ch = wide.tile([H, csz], bf16, tag="pre", bufs=2)
            n_ch = wide.tile([H, csz], bf16, tag="n", bufs=2)
            zh_ch = wide.tile([H, csz], bf16, tag="zh", bufs=2)
            z_ch = wide.tile([H, csz], bf16, tag="z", bufs=2)

            for t0 in range(0, csz, NTILE):
                tsz = min(NTILE, csz - t0)
                sl = slice(t0, t0 + tsz)
                p_r = psum.tile([H, tsz], f32, tag="p_r")
                p_z = psum.tile([H, tsz], f32, tag="p_z")
                p_in = psum.tile([H, tsz], f32, tag="p_in")
                p_hn = psum.tile([H, tsz], f32, tag="p_hn")

                # gate pre-activations, 512 fp32 per matmul (one PSUM bank)
                for q0 in range(0, tsz, 512):
                    qs = slice(t0 + q0, t0 + q0 + 512)
                    qd = slice(q0, q0 + 512)
                    nc.tensor.matmul(p_r[:, qd], w_ih_sb[:, 0:H], x_sb[:, qs],
                                     start=True, stop=False)
                    nc.tensor.matmul(p_r[:, qd], w_hh_sb[:, 0:H], h_sb[:, qs],
                                     start=False, stop=True)
                    nc.tensor.matmul(p_z[:, qd], w_ih_sb[:, H : 2 * H],
                                     x_sb[:, qs], start=True, stop=False)
                    nc.tensor.matmul(p_z[:, qd], w_hh_sb[:, H : 2 * H],
                                     h_sb[:, qs], start=False, stop=True)
                    nc.tensor.matmul(p_in[:, qd], w_ih_sb[:, 2 * H : 3 * H],
                                     x_sb[:, qs], start=True, stop=True)
                    nc.tensor.matmul(p_hn[:, qd], w_hh_sb[:, 2 * H : 3 * H],
                                     h_sb[:, qs], start=True, stop=True)

                r_t = mids.tile([H, tsz], bf16, tag="r")
                nc.scalar.activation(out=r_t, in_=p_r, func=sig,
                                     bias=b_sb[:, 0:1], scale=1.0)
                nc.scalar.activation(out=z_ch[:, sl], in_=p_z, func=sig,
                                     bias=b_sb[:, 1:2], scale=1.0)
                # zh = z*h needs only z and h -> runs on GpSimd in parallel
                # with the whole STT/pre/tanh chain (off the critical path)
                nc.gpsimd.tensor_mul(out=zh_ch[:, sl], in0=z_ch[:, sl],
                                     in1=h_sb[:, sl])

                # t = (h_n + b_hn) * r, fused on DVE; pre = t + i_n
                t_t = mids.tile([H, tsz], bf16, tag="t")
                nc.vector.scalar_tensor_tensor(
                    out=t_t, in0=p_hn, scalar=b_sb[:, 2:3], in1=r_t,
                    op0=add_op, op1=mult_op)
                nc.vector.tensor_add(out=pre_ch[:, sl], in0=t_t, in1=p_in)

            # one chunk-wide tanh (saves one ACT op + sem per tile on the
            # saturated pacing engine)
            nc.scalar.activation(out=n_ch, in_=pre_ch, func=tanh,
                                 bias=b_sb[:, 3:4], scale=1.0)

            # out = n + z*(h-n) = zh - (z-1)*n, chunk-wide behind the tanh
            # (fewer ops on DVE, the pacing engine once tanh is batched)
            v_ch = wide.tile([H, csz], bf16, tag="v", bufs=2)
            nc.vector.scalar_tensor_tensor(
                out=v_ch, in0=z_ch, scalar=1.0, in1=n_ch,
                op0=mybir.AluOpType.subtract, op1=mult_op)
            nc.vector.tensor_sub(out=o_sb, in0=zh_ch, in1=v_ch)

            nc.sync.dma_start(out=outT[:, c0 : c0 + csz], in_=o_sb)

    _patch_json(nc)
    return nc


def _get_nc(bpc: int) -> bass.Bass:
    if bpc not in _NC_CACHE:
        _NC_CACHE[bpc] = _build_nc(bpc)
    return _NC_CACHE[bpc]


def kernel(node_ids, messages, memory, W_ih, W_hh, b_ih, b_hh):
    global LAST_RESULT
    node_ids = np.asarray(node_ids)
    messages = np.asarray(messages, dtype=np.float32)
    memory = np.asarray(memory, dtype=np.float32)
    W_ih = np.asarray(W_ih, dtype=np.float32)
    W_hh = np.asarray(W_hh, dtype=np.float32)
    b_ih = np.asarray(b_ih, dtype=np.float32)
    b_hh = np.asarray(b_hh, dtype=np.float32)

    B = node_ids.shape[0]
    per = -(-B // N_CORES)                       # rows per core (unpadded)
    bpc = -(-per // 512) * 512                   # padded to 512 multiple
    nc = _get_nc(bpc)

    current = memory[node_ids]                   # [B, H] host gather

    w_ihT = np.ascontiguousarray(W_ih.T).astype(BF16)
    w_hhT = np.ascontiguousarray(W_hh.T).astype(BF16)
    bias = np.empty((H, 4), dtype=np.float32)
    bias[:, 0] = b_ih[0:H] + b_hh[0:H]
    bias[:, 1] = b_ih[H : 2 * H] + b_hh[H : 2 * H]
    bias[:, 2] = b_hh[2 * H : 3 * H]
    bias[:, 3] = b_ih[2 * H : 3 * H]

    in_maps = []
    for c in range(N_CORES):
        lo = c * per
        hi = min(lo + per, B)
        xT = np.zeros((H, bpc), dtype=BF16)
        hT = np.zeros((H, bpc), dtype=BF16)
        if hi > lo:
            xT[:, : hi - lo] = messages[lo:hi].T
            hT[:, : hi - lo] = current[lo:hi].T
        in_maps.append({
            "xT": xT, "hT": hT,
            "w_ihT": w_ihT, "w_hhT": w_hhT, "biases": bias,
        })

    res = run_bass_kernel_spmd(nc, in_maps, list(range(N_CORES)))
    LAST_RESULT = res

    updated = np.empty((B, H), dtype=np.float32)
    for c in range(N_CORES):
        lo = c * per
        hi = min(lo + per, B)
        if hi > lo:
            updated[lo:hi] = res.results[c]["outT"][:, : hi - lo].T.astype(np.float32)

    new_memory = memory.copy()
    new_memory[node_ids] = updated
    return new_memory

